# revision 17
# baseline (speedup 1.0000x reference)
"""Causal single-head attention (B=4, S=2048, D=DK=1024) on 8 trn2 NeuronCores.

Sharding: data-parallel over batch x interleaved q-blocks. Core c handles
batch b=c//2, parity p=c%2, owning the 8 q-blocks {2j+p : j in 0..7} (128 rows
each). One uniform SPMD program runs on all 8 cores; per-core differences are
carried entirely by the input data (host-side column permutation + mask tile).

v2 design (vs v1):
- Position-interleaved context layout: column/row block order is
  [own_0, other_0, own_1, other_1, ...] so q-tile j's causal context is the
  contiguous prefix of 2j+2 blocks. Scores run as contiguous 512-col chunks
  and the additive causal mask (madd) is a single j-invariant [128, 256] tile
  applied to the last two blocks.
- Whole score/P/out path in fp16 (tf32-grade mantissa): xct/wqk/gt/attn/xc/
  p/pt/wv all fp16, halving input DMA (12.1MB total) and keeping every matmul
  at 1 cycle/row with FWL-eligible weight loads. PSUM accumulation is fp32.
- Software-pipelined j-loop (order 1,2,..,7,0): scores for the next tile are
  emitted before softmax/P/out of the current one so the PE never idles on
  the softmax chain; the smallest tile runs last to shrink the serial tail.
- Softmax stats (global max, sum, reciprocal) on GpSimd, exp on Scalar,
  PSUM->SBUF copies + segment maxes on Vector: no engine-queue coupling.
- Transposes batched into full-bank PSUM tiles (8 slots) with one grouped
  DVE copy per group; two banks ping-pong so PE never waits on a copy.
- Dependency-free fp16 warmup matmuls from ~1.5us keep the PE HAM-warm
  before the first DMA lands; out DMAs ride the idle GpSimd queue.

Math per core (W_QK = W_Q W_K^T folded on host):
    G^T = W_QK^T X_q^T                 [dk, 1024]   (q = own 8 blocks)
    S   = G X_ctx^T   (contiguous causal prefix, compact layout)
    A   = softmax(S/32 with -1e9 madd pre-scale)    [fp16]
    P   = A X_ctx                       (fp16 operands, fp32 PSUM)
    out = (P W_V) * rcp                 (then scatter rows back on host)
"""

import numpy as np

B, S, D = 4, 2048, 1024
P = 128               # partitions
NJ = 8                # q-tiles per core
NCORES = 8
MASK_FILL = -1.0e9
JORDER = [1, 2, 3, 4, 5, 6, 7, 0]

_cache = {}


def _build_program():
    from contextlib import ExitStack
    import concourse.bass as bass
    import concourse.bacc as bacc
    import concourse.tile as tile
    import concourse.mybir as mybir
    from concourse import masks

    f32 = mybir.dt.float32
    fp16 = mybir.dt.float16
    Exp = mybir.ActivationFunctionType.Exp
    Copy = mybir.ActivationFunctionType.Copy
    AX = mybir.AxisListType.X
    ts = bass.ts

    nc = bacc.Bacc("TRN2", target_bir_lowering=False, debug=False,
                   enable_asserts=False)

    xct_d = nc.dram_tensor("xct", [D, S], fp16, kind="ExternalInput").ap()
    xc_d = nc.dram_tensor("xc", [S, D], fp16, kind="ExternalInput").ap()
    wqk_d = nc.dram_tensor("wqk", [D, D], fp16, kind="ExternalInput").ap()
    wv_d = nc.dram_tensor("wv", [D, D], fp16, kind="ExternalInput").ap()
    madd_d = nc.dram_tensor("madd", [P, 2 * P], f32, kind="ExternalInput").ap()
    out_d = nc.dram_tensor("out", [NJ * P, D], f32, kind="ExternalOutput").ap()

    # DRAM xct columns come host-packed as [1024 even-block | 1024 odd-block]
    # so every DMA partition line is a contiguous 2KB run; the SBUF-side AP
    # scatters them into the interleaved (pair, parity) layout.
    xct_r = xct_d.rearrange("(c p) (s b k) -> c p s b k",
                            p=P, s=2, k=P)              # [8, 128, 2, 8, 128]
    xc_r = xc_d.rearrange("(b p) d -> p b d", p=P)      # [128, 16, 1024]
    wqk_r = wqk_d.rearrange("(c p) n -> c p n", p=P)    # [8, 128, 1024]
    wv_r = wv_d.rearrange("(c p) n -> p c n", p=P)      # [128, 8, 1024]

    with tile.TileContext(nc) as tc, ExitStack() as es:
        # ---- persistent pools -------------------------------------------
        perm = es.enter_context(tc.tile_pool(name="perm", bufs=1))
        xct_sb = perm.tile([P, 8, 8, 2, P], fp16)   # X^T (dc, pair, par, col)
        gt_sb = perm.tile([P, 8, 1024], fp16)       # G^T (dt, q)
        xc_sb = perm.tile([P, 16, 1024], fp16)      # X rows (pos, d)
        wv_sb = perm.tile([P, 8, 1024], fp16)
        madd_sb = perm.tile([P, 2 * P], f32)
        ident = perm.tile([P, P], fp16)

        masks.make_identity(nc, ident[:])

        # pools that outlive phase G
        statp = tc.alloc_tile_pool(name="stats", bufs=2)
        earlyp = tc.alloc_tile_pool(name="early", bufs=1)
        workp = tc.alloc_tile_pool(name="work", bufs=2)
        srows = [earlyp.tile([P, 2048], f32, tag=f"srow{i}",
                             name=f"srow{i}") for i in range(2)]
        attns = [earlyp.tile([P, 2048], fp16, tag=f"attn{i}",
                             name=f"attn{i}") for i in range(2)]
        attnT = earlyp.tile([P, 2048], fp16, tag="attnT")
        p_sb = earlyp.tile([P, 1024], fp16, tag="p")
        pt_sb = earlyp.tile([P, 1024], fp16, tag="pt")

        # ---- input DMAs issued in first-use order; then phase G ---------
        nc.sync.dma_start(madd_sb[:], madd_d)

        # ---- phase G: G^T = (W_QK^T X_q^T) ------------------------------
        # dc-outer over 8 live PSUM banks: matmul (dc) only needs wqk/xct
        # chunk dc, so compute tracks the DMA stream chunk-by-chunk instead
        # of blocking each dt-group on the last chunk's arrival.
        with tc.tile_pool(name="wqk", bufs=1) as wqkp, \
             tc.tile_pool(name="pps", bufs=8, space="PSUM") as pps:
            # HAM warm-up: dependency-free fp16 matmuls on (not yet written)
            # attn garbage keep the PE busy from ~1.5us so G starts warm.
            warm = pps.tile([P, 512], f32, tag="ps", name="warmup")
            for _ in range(16):
                nc.tensor.matmul(warm[:], attns[0][:, 0:P],
                                 attns[0][:, 0:512])
            wqk_sb = wqkp.tile([P, 8, 1024], fp16)
            for dc in range(8):
                # wqk on the (idle) scalar DMA queue, evens on sync: two DGE
                # rings generate descriptors in parallel
                nc.scalar.dma_start(wqk_sb[:, dc, :], wqk_r[dc])
                nc.sync.dma_start(xct_sb[:, dc, :, 0, :],
                                  xct_r[dc, :, 0, :, :])
            for h in (0, 1):
                psl = {dt: pps.tile([P, 512], f32, tag="ps",
                                    name=f"psG{dt}{h}")
                       for dt in range(8)}
                for dc in range(8):
                    for dt in range(8):
                        nc.tensor.matmul(
                            psl[dt][:], wqk_sb[:, dc, ts(dt, P)],
                            xct_sb[:, dc, 4 * h:4 * h + 4, 0, :],
                            start=(dc == 0), stop=(dc == 7))
                for dt in range(8):
                    nc.scalar.copy(gt_sb[:, dt, 512 * h:512 * h + 512],
                                   psl[dt][:])

        # phase-D inputs in first-use order (j order 1,2,..,7,0): odd-block
        # pairs 0-3 first (scores 1-3), then xc/wv for P_1/out_1, then the
        # rest
        for dc in range(8):
            nc.sync.dma_start(xct_sb[:, dc, 0:4, 1, :],
                              xct_r[dc, :, 1, 0:4, :])
        nc.sync.dma_start(xc_sb[:, 0:4, :], xc_r[:, 0:4, :])
        for d0 in (0, 4):
            nc.scalar.dma_start(wv_sb[:, d0:d0 + 4, :], wv_r[:, d0:d0 + 4, :])
        for dc in range(8):
            nc.sync.dma_start(xct_sb[:, dc, 4:8, 1, :],
                              xct_r[dc, :, 1, 4:8, :])
        nc.sync.dma_start(xc_sb[:, 4:8, :], xc_r[:, 4:8, :])
        nc.sync.dma_start(xc_sb[:, 8:12, :], xc_r[:, 8:12, :])
        nc.sync.dma_start(xc_sb[:, 12:16, :], xc_r[:, 12:16, :])

        # ---- phase D: software-pipelined attention ----------------------
        spsp = tc.alloc_tile_pool(name="sps", bufs=2, space="PSUM")
        trp = tc.alloc_tile_pool(name="trp", bufs=1, space="PSUM")
        ppp = tc.alloc_tile_pool(name="ppp", bufs=1, space="PSUM")
        opsp = tc.alloc_tile_pool(name="ops", bufs=1, space="PSUM")
        trt = [trp.tile([P, 8, P], fp16, tag=f"tr{i}", name=f"tr{i}")
               for i in range(2)]

        def emit_scores(j):
            """Score matmuls + per-chunk PSUM->srow copies and maxes."""
            srow = srows[j % 2]
            npr = j + 1
            nch = (npr + 1) // 2
            mx = statp.tile([P, 8], f32, tag=f"mx{j % 2}", name=f"mx{j}")
            for ch in range(nch):
                pr = 2 * ch
                cp = min(2, npr - pr)
                w = cp * 256
                off = pr * 256
                ps = spsp.tile([P, 512], f32, tag="ps", name=f"s{j}c{ch}")
                for dc in range(8):
                    nc.tensor.matmul(ps[:, :w], gt_sb[:, dc, ts(j, P)],
                                     xct_sb[:, dc, pr:pr + cp, :, :],
                                     start=(dc == 0), stop=(dc == 7))
                if ch == nch - 1:
                    if w == 512:
                        nc.vector.tensor_copy(srow[:, off:off + 256],
                                              ps[:, 0:256])
                    nc.vector.tensor_add(srow[:, off + w - 256:off + w],
                                         ps[:, w - 256:w], madd_sb[:])
                    nc.vector.reduce_max(mx[:, ch:ch + 1],
                                         srow[:, off:off + w], axis=AX)
                else:
                    nc.vector.tensor_copy(srow[:, off:off + w], ps[:, :w])
                    nc.vector.reduce_max(mx[:, ch:ch + 1], ps[:, :w], axis=AX)
            return mx, nch

        def emit_stats(j, mx, nch):
            """Global (negated, pre-scaled) max — emitted before the next
            tile's score chain so it doesn't queue behind it on DVE."""
            nmx = statp.tile([P, 1], f32, tag=f"nmx{j % 2}", name=f"nmx{j}")
            nc.vector.reduce_max(nmx[:], mx[:, :nch], axis=AX, negate=True)
            nc.vector.tensor_scalar_mul(nmx[:], nmx[:], 1.0 / 32.0)
            return nmx

        def emit_rest(j, nch, nmx):
            srow, attn = srows[j % 2], attns[j % 2]
            nk = 2 * (j + 1)
            W = nk * P
            seseg = statp.tile([P, 8], f32, tag=f"se{j % 2}", name=f"se{j}")
            for ch in range(nch):
                off = 512 * ch
                w = min(512, W - off)
                nc.scalar.activation(attn[:, off:off + w],
                                     srow[:, off:off + w], Exp,
                                     bias=nmx[:], scale=1.0 / 32.0,
                                     accum_out=seseg[:, ch:ch + 1])
            sumexp = statp.tile([P, 1], f32, tag=f"sum{j % 2}",
                                name=f"sum{j}")
            nc.vector.reduce_sum(sumexp[:], seseg[:, :nch], axis=AX)
            rcp = statp.tile([P, 1], f32, tag=f"rcp{j % 2}", name=f"rcp{j}")
            nc.vector.reciprocal(rcp[:], sumexp[:])

            # A^T via batched PE transposes (full-bank groups of 8)
            for gi, g0 in enumerate(range(0, nk, 8)):
                gn = min(8, nk - g0)
                tr = trt[gi % 2]
                for i in range(gn):
                    nc.tensor.transpose(tr[:, i, :], attn[:, ts(g0 + i, P)],
                                        ident[:])
                nc.vector.tensor_copy(attnT[:, g0 * P:(g0 + gn) * P],
                                      tr[:, :gn, :])
            # P = A X_ctx
            pp0 = ppp.tile([P, 512], f32, tag="pp0", name="pp0")
            pp1 = ppp.tile([P, 512], f32, tag="pp1", name="pp1")
            for c in range(nk):
                for pp, dh in ((pp0, 0), (pp1, 512)):
                    nc.tensor.matmul(pp[:], attnT[:, ts(c, P)],
                                     xc_sb[:, c, dh:dh + 512],
                                     start=(c == 0), stop=(c == nk - 1))
            nc.vector.tensor_copy(p_sb[:, 0:512], pp0[:])
            nc.vector.tensor_copy(p_sb[:, 512:1024], pp1[:])

            # P^T via batched transposes (two half-bank groups of 4)
            for gi, g0 in enumerate((0, 4)):
                tr = trt[gi % 2]
                for i in range(4):
                    nc.tensor.transpose(tr[:, i, :], p_sb[:, ts(g0 + i, P)],
                                        ident[:])
                nc.vector.tensor_copy(pt_sb[:, g0 * P:(g0 + 4) * P],
                                      tr[:, 0:4, :])
            # out = (P W_V) * rcp — half-outer so half 0's normalize + DMA
            # drain under half 1's matmuls (shrinks the last tile's tail)
            op0 = opsp.tile([P, 512], f32, tag="op0", name="op0")
            op1 = opsp.tile([P, 512], f32, tag="op1", name="op1")
            out_sb = workp.tile([P, 1024], f32, tag="out", name=f"out{j}")
            for op, dh in ((op0, 0), (op1, 512)):
                for dc in range(8):
                    nc.tensor.matmul(op[:], pt_sb[:, ts(dc, P)],
                                     wv_sb[:, dc, dh:dh + 512],
                                     start=(dc == 0), stop=(dc == 7))
                nc.scalar.activation(out_sb[:, dh:dh + 512], op[:], Copy,
                                     scale=rcp[:])
                nc.gpsimd.dma_start(out_d[ts(j, P), dh:dh + 512],
                                    out_sb[:, dh:dh + 512])

        pend = emit_scores(JORDER[0])
        for idx, j in enumerate(JORDER):
            mx, nch = pend
            nmx = emit_stats(j, mx, nch)
            nxt_pend = emit_scores(JORDER[idx + 1]) if idx < NJ - 1 else None
            emit_rest(j, nch, nmx)
            pend = nxt_pend

        opsp.release()
        ppp.release()
        trp.release()
        workp.release()
        earlyp.release()
        statp.release()
        spsp.release()

    nc.compile()
    return nc


def _prep_inputs(sequence_repr, W_Q, W_K, W_V, mask):
    """Build the 8 per-core input dicts (host-side slicing/permutation)."""
    wqk = np.ascontiguousarray(W_Q @ W_K.T).astype(np.float16)
    wv = np.ascontiguousarray(W_V).astype(np.float16)
    in_maps = []
    meta = []
    for c in range(NCORES):
        b, par = divmod(c, 2)
        pos_blocks = []
        for j in range(NJ):
            pos_blocks += [2 * j + par, 2 * j + 1 - par]
        rows_perm = np.concatenate(
            [np.arange(g * P, (g + 1) * P) for g in pos_blocks])
        xb = sequence_repr[b]
        # xct columns packed [all even-pos blocks | all odd-pos blocks] for
        # 2KB-contiguous DMA lines; xc rows stay position-interleaved
        halves_perm = np.concatenate(
            [np.arange(g * P, (g + 1) * P)
             for g in pos_blocks[0::2] + pos_blocks[1::2]])
        xct = np.ascontiguousarray(xb.T[:, halves_perm]).astype(np.float16)
        xc = np.ascontiguousarray(xb[rows_perm]).astype(np.float16)
        # j-invariant boundary mask: cols [0:128) = own (diagonal) block,
        # [128:256) = other-parity neighbour (all-masked or all-allowed)
        g0, gb0 = par, 1 - par
        qr0 = slice(g0 * P, g0 * P + P)
        madd = np.empty((P, 2 * P), np.float32)
        madd[:, 0:P] = np.where(mask[b, qr0, g0 * P:(g0 + 1) * P],
                                0.0, MASK_FILL)
        madd[:, P:2 * P] = np.where(mask[b, qr0, gb0 * P:(gb0 + 1) * P],
                                    0.0, MASK_FILL)
        in_maps.append({"xct": xct, "xc": xc, "wqk": wqk, "wv": wv,
                        "madd": madd})
        qrows = np.concatenate(
            [np.arange((2 * j + par) * P, (2 * j + par + 1) * P)
             for j in range(NJ)])
        meta.append((b, qrows))
    return in_maps, meta


def run(sequence_repr, W_Q, W_K, W_V, mask, trace=False):
    from concourse.bass_utils import run_bass_kernel_spmd

    if "nc" not in _cache:
        _cache["nc"] = _build_program()
    nc = _cache["nc"]
    in_maps, meta = _prep_inputs(
        np.asarray(sequence_repr, np.float32), np.asarray(W_Q, np.float32),
        np.asarray(W_K, np.float32), np.asarray(W_V, np.float32),
        np.asarray(mask))
    res = run_bass_kernel_spmd(nc, in_maps, core_ids=list(range(NCORES)),
                               trace=trace)
    out = np.empty((B, S, D), np.float32)
    for c in range(NCORES):
        b, qrows = meta[c]
        out[b, qrows] = res.results[c]["out"]
    return out, res


def kernel(**inputs):
    out, _ = run(**inputs)
    return out


# revision 19
# speedup vs baseline: 1.0662x; 1.0662x over previous
"""Causal single-head attention (B=4, S=2048, D=DK=1024) on 8 trn2 NeuronCores.

Sharding: data-parallel over batch x interleaved q-blocks. Core c handles
batch b=c//2, parity p=c%2, owning the 8 q-blocks {2j+p : j in 0..7} (128 rows
each). One uniform SPMD program runs on all 8 cores; per-core differences are
carried entirely by the input data (host-side column permutation + mask tile).

v2 design (vs v1):
- Position-interleaved context layout: column/row block order is
  [own_0, other_0, own_1, other_1, ...] so q-tile j's causal context is the
  contiguous prefix of 2j+2 blocks. Scores run as contiguous 512-col chunks
  and the additive causal mask (madd) is a single j-invariant [128, 256] tile
  applied to the last two blocks.
- Whole score/P/out path in fp16 (tf32-grade mantissa): xct/wqk/gt/attn/xc/
  p/pt/wv all fp16, halving input DMA (12.1MB total) and keeping every matmul
  at 1 cycle/row with FWL-eligible weight loads. PSUM accumulation is fp32.
- Software-pipelined j-loop (order 1,2,..,7,0): scores for the next tile are
  emitted before softmax/P/out of the current one so the PE never idles on
  the softmax chain; the smallest tile runs last to shrink the serial tail.
- Softmax stats (global max, sum, reciprocal) on GpSimd, exp on Scalar,
  PSUM->SBUF copies + segment maxes on Vector: no engine-queue coupling.
- Transposes batched into full-bank PSUM tiles (8 slots) with one grouped
  DVE copy per group; two banks ping-pong so PE never waits on a copy.
- Dependency-free fp16 warmup matmuls from ~1.5us keep the PE HAM-warm
  before the first DMA lands; out DMAs ride the idle GpSimd queue.

Math per core (W_QK = W_Q W_K^T folded on host):
    G^T = W_QK^T X_q^T                 [dk, 1024]   (q = own 8 blocks)
    S   = G X_ctx^T   (contiguous causal prefix, compact layout)
    A   = softmax(S/32 with -1e9 madd pre-scale)    [fp16]
    P   = A X_ctx                       (fp16 operands, fp32 PSUM)
    out = (P W_V) * rcp                 (then scatter rows back on host)
"""

import numpy as np

B, S, D = 4, 2048, 1024
P = 128               # partitions
NJ = 8                # q-tiles per core
NCORES = 8
MASK_FILL = -1.0e9
JORDER = [1, 2, 3, 4, 5, 6, 7, 0]

_cache = {}


def _build_program():
    from contextlib import ExitStack
    import concourse.bass as bass
    import concourse.bacc as bacc
    import concourse.tile as tile
    import concourse.mybir as mybir
    from concourse import masks

    f32 = mybir.dt.float32
    fp16 = mybir.dt.float16
    Exp = mybir.ActivationFunctionType.Exp
    Copy = mybir.ActivationFunctionType.Copy
    AX = mybir.AxisListType.X
    ts = bass.ts

    nc = bacc.Bacc("TRN2", target_bir_lowering=False, debug=False,
                   enable_asserts=False)

    xct_d = nc.dram_tensor("xct", [D, S], fp16, kind="ExternalInput").ap()
    xc_d = nc.dram_tensor("xc", [S, D], fp16, kind="ExternalInput").ap()
    wqk_d = nc.dram_tensor("wqk", [D, D], fp16, kind="ExternalInput").ap()
    wv_d = nc.dram_tensor("wv", [D, D], fp16, kind="ExternalInput").ap()
    madd_d = nc.dram_tensor("madd", [P, 2 * P], f32, kind="ExternalInput").ap()
    out_d = nc.dram_tensor("out", [NJ * P, D], f32, kind="ExternalOutput").ap()

    # DRAM xct columns come host-packed as [1024 even-block | 1024 odd-block]
    # so every DMA partition line is a contiguous 2KB run; the SBUF-side AP
    # scatters them into the interleaved (pair, parity) layout.
    xct_r = xct_d.rearrange("(c p) (s b k) -> c p s b k",
                            p=P, s=2, k=P)              # [8, 128, 2, 8, 128]
    xc_r = xc_d.rearrange("(b p) d -> p b d", p=P)      # [128, 16, 1024]
    wqk_r = wqk_d.rearrange("(c p) n -> c p n", p=P)    # [8, 128, 1024]
    wv_r = wv_d.rearrange("(c p) n -> p c n", p=P)      # [128, 8, 1024]

    with tile.TileContext(nc) as tc, ExitStack() as es:
        # ---- persistent pools -------------------------------------------
        perm = es.enter_context(tc.tile_pool(name="perm", bufs=1))
        xct_sb = perm.tile([P, 8, 8, 2, P], fp16)   # X^T (dc, pair, par, col)
        gt_sb = perm.tile([P, 8, 1024], fp16)       # G^T (dt, q)
        xc_sb = perm.tile([P, 16, 1024], fp16)      # X rows (pos, d)
        wv_sb = perm.tile([P, 8, 1024], fp16)
        madd_sb = perm.tile([P, 2 * P], f32)
        ident = perm.tile([P, P], fp16)

        masks.make_identity(nc, ident[:])

        # pools that outlive phase G
        statp = tc.alloc_tile_pool(name="stats", bufs=2)
        earlyp = tc.alloc_tile_pool(name="early", bufs=1)
        workp = tc.alloc_tile_pool(name="work", bufs=2)
        srows = [earlyp.tile([P, 2048], f32, tag=f"srow{i}",
                             name=f"srow{i}") for i in range(2)]
        attns = [earlyp.tile([P, 2048], fp16, tag=f"attn{i}",
                             name=f"attn{i}") for i in range(2)]
        attnT = earlyp.tile([P, 2048], fp16, tag="attnT")
        p_sb = earlyp.tile([P, 1024], fp16, tag="p")
        pt_sb = earlyp.tile([P, 1024], fp16, tag="pt")

        # ---- input DMAs issued in first-use order; then phase G ---------
        nc.sync.dma_start(madd_sb[:], madd_d)

        # ---- phase G: G^T = (W_QK^T X_q^T) ------------------------------
        # dc-outer over 8 live PSUM banks: matmul (dc) only needs wqk/xct
        # chunk dc, so compute tracks the DMA stream chunk-by-chunk instead
        # of blocking each dt-group on the last chunk's arrival.
        with tc.tile_pool(name="wqk", bufs=1) as wqkp, \
             tc.tile_pool(name="pps", bufs=8, space="PSUM") as pps:
            # HAM warm-up: dependency-free fp16 matmuls on (not yet written)
            # attn garbage keep the PE busy from ~1.5us so G starts warm.
            warm = pps.tile([P, 512], f32, tag="ps", name="warmup")
            for _ in range(14):
                nc.tensor.matmul(warm[:], attns[0][:, 0:P],
                                 attns[0][:, 0:512])
            wqk_sb = wqkp.tile([P, 8, 1024], fp16)
            # pass 0 needs only (wqk[dc], even pairs 0-3 of dc): 384KB per
            # dc-step keeps pass 0 compute-bound on the DMA stream; the
            # second even half lands during pass 0 so pass 1 never waits
            for dc in range(8):
                nc.sync.dma_start(wqk_sb[:, dc, :], wqk_r[dc])
                nc.sync.dma_start(xct_sb[:, dc, 0:4, 0, :],
                                  xct_r[dc, :, 0, 0:4, :])
            for dc in range(8):
                nc.sync.dma_start(xct_sb[:, dc, 4:8, 0, :],
                                  xct_r[dc, :, 0, 4:8, :])
            for h in (0, 1):
                psl = {dt: pps.tile([P, 512], f32, tag="ps",
                                    name=f"psG{dt}{h}")
                       for dt in range(8)}
                for dc in range(8):
                    for dt in range(8):
                        nc.tensor.matmul(
                            psl[dt][:], wqk_sb[:, dc, ts(dt, P)],
                            xct_sb[:, dc, 4 * h:4 * h + 4, 0, :],
                            start=(dc == 0), stop=(dc == 7))
                for dt in range(8):
                    nc.scalar.copy(gt_sb[:, dt, 512 * h:512 * h + 512],
                                   psl[dt][:])

        # phase-D inputs in first-use order (j order 1,2,..,7,0): odd-block
        # pairs 0-3 first (scores 1-3), then wv/xc for out_1/P_1, then the
        # rest
        for dc in range(8):
            nc.sync.dma_start(xct_sb[:, dc, 0:4, 1, :],
                              xct_r[dc, :, 1, 0:4, :])
        nc.sync.dma_start(wv_sb[:, 0:4, :], wv_r[:, 0:4, :])
        nc.sync.dma_start(xc_sb[:, 0:4, :], xc_r[:, 0:4, :])
        nc.sync.dma_start(wv_sb[:, 4:8, :], wv_r[:, 4:8, :])
        for dc in range(8):
            nc.sync.dma_start(xct_sb[:, dc, 4:8, 1, :],
                              xct_r[dc, :, 1, 4:8, :])
        nc.sync.dma_start(xc_sb[:, 4:8, :], xc_r[:, 4:8, :])
        nc.sync.dma_start(xc_sb[:, 8:12, :], xc_r[:, 8:12, :])
        nc.sync.dma_start(xc_sb[:, 12:16, :], xc_r[:, 12:16, :])

        # ---- phase D: software-pipelined attention ----------------------
        spsp = tc.alloc_tile_pool(name="sps", bufs=2, space="PSUM")
        trp = tc.alloc_tile_pool(name="trp", bufs=1, space="PSUM")
        ppp = tc.alloc_tile_pool(name="ppp", bufs=1, space="PSUM")
        opsp = tc.alloc_tile_pool(name="ops", bufs=1, space="PSUM")
        trt = [trp.tile([P, 8, P], fp16, tag=f"tr{i}", name=f"tr{i}")
               for i in range(2)]

        def emit_scores(j):
            """Score matmuls + per-chunk PSUM->srow copies and maxes."""
            srow = srows[j % 2]
            npr = j + 1
            nch = (npr + 1) // 2
            mx = statp.tile([P, 8], f32, tag=f"mx{j % 2}", name=f"mx{j}")
            for ch in range(nch):
                pr = 2 * ch
                cp = min(2, npr - pr)
                w = cp * 256
                off = pr * 256
                ps = spsp.tile([P, 512], f32, tag="ps", name=f"s{j}c{ch}")
                for dc in range(8):
                    nc.tensor.matmul(ps[:, :w], gt_sb[:, dc, ts(j, P)],
                                     xct_sb[:, dc, pr:pr + cp, :, :],
                                     start=(dc == 0), stop=(dc == 7))
                if ch == nch - 1:
                    if w == 512:
                        nc.vector.tensor_copy(srow[:, off:off + 256],
                                              ps[:, 0:256])
                    nc.vector.tensor_add(srow[:, off + w - 256:off + w],
                                         ps[:, w - 256:w], madd_sb[:])
                    nc.vector.reduce_max(mx[:, ch:ch + 1],
                                         srow[:, off:off + w], axis=AX)
                else:
                    nc.vector.tensor_copy(srow[:, off:off + w], ps[:, :w])
                    nc.vector.reduce_max(mx[:, ch:ch + 1], ps[:, :w], axis=AX)
            return mx, nch

        def emit_stats(j, mx, nch):
            """Global (negated, pre-scaled) max — emitted before the next
            tile's score chain so it doesn't queue behind it on DVE."""
            nmx = statp.tile([P, 1], f32, tag=f"nmx{j % 2}", name=f"nmx{j}")
            nc.vector.reduce_max(nmx[:], mx[:, :nch], axis=AX, negate=True)
            nc.vector.tensor_scalar_mul(nmx[:], nmx[:], 1.0 / 32.0)
            return nmx

        def emit_rest(j, nch, nmx):
            srow, attn = srows[j % 2], attns[j % 2]
            nk = 2 * (j + 1)
            W = nk * P
            seseg = statp.tile([P, 8], f32, tag=f"se{j % 2}", name=f"se{j}")
            for ch in range(nch):
                off = 512 * ch
                w = min(512, W - off)
                nc.scalar.activation(attn[:, off:off + w],
                                     srow[:, off:off + w], Exp,
                                     bias=nmx[:], scale=1.0 / 32.0,
                                     accum_out=seseg[:, ch:ch + 1])
            sumexp = statp.tile([P, 1], f32, tag=f"sum{j % 2}",
                                name=f"sum{j}")
            nc.vector.reduce_sum(sumexp[:], seseg[:, :nch], axis=AX)
            rcp = statp.tile([P, 1], f32, tag=f"rcp{j % 2}", name=f"rcp{j}")
            nc.vector.reciprocal(rcp[:], sumexp[:])

            # A^T via batched PE transposes (full-bank groups of 8)
            for gi, g0 in enumerate(range(0, nk, 8)):
                gn = min(8, nk - g0)
                tr = trt[gi % 2]
                for i in range(gn):
                    nc.tensor.transpose(tr[:, i, :], attn[:, ts(g0 + i, P)],
                                        ident[:])
                nc.vector.tensor_copy(attnT[:, g0 * P:(g0 + gn) * P],
                                      tr[:, :gn, :])
            # P = A X_ctx
            pp0 = ppp.tile([P, 512], f32, tag="pp0", name="pp0")
            pp1 = ppp.tile([P, 512], f32, tag="pp1", name="pp1")
            for c in range(nk):
                for pp, dh in ((pp0, 0), (pp1, 512)):
                    nc.tensor.matmul(pp[:], attnT[:, ts(c, P)],
                                     xc_sb[:, c, dh:dh + 512],
                                     start=(c == 0), stop=(c == nk - 1))
            nc.vector.tensor_copy(p_sb[:, 0:512], pp0[:])
            nc.vector.tensor_copy(p_sb[:, 512:1024], pp1[:])

            # P^T via batched transposes (two half-bank groups of 4)
            for gi, g0 in enumerate((0, 4)):
                tr = trt[gi % 2]
                for i in range(4):
                    nc.tensor.transpose(tr[:, i, :], p_sb[:, ts(g0 + i, P)],
                                        ident[:])
                nc.vector.tensor_copy(pt_sb[:, g0 * P:(g0 + 4) * P],
                                      tr[:, 0:4, :])
            # out = (P W_V) * rcp — half-outer so half 0's normalize + DMA
            # drain under half 1's matmuls (shrinks the last tile's tail)
            op0 = opsp.tile([P, 512], f32, tag="op0", name="op0")
            op1 = opsp.tile([P, 512], f32, tag="op1", name="op1")
            out_sb = workp.tile([P, 1024], f32, tag="out", name=f"out{j}")
            for op, dh in ((op0, 0), (op1, 512)):
                for dc in range(8):
                    nc.tensor.matmul(op[:], pt_sb[:, ts(dc, P)],
                                     wv_sb[:, dc, dh:dh + 512],
                                     start=(dc == 0), stop=(dc == 7))
                nc.scalar.activation(out_sb[:, dh:dh + 512], op[:], Copy,
                                     scale=rcp[:])
                nc.gpsimd.dma_start(out_d[ts(j, P), dh:dh + 512],
                                    out_sb[:, dh:dh + 512])

        pend = emit_scores(JORDER[0])
        for idx, j in enumerate(JORDER):
            mx, nch = pend
            nmx = emit_stats(j, mx, nch)
            nxt_pend = emit_scores(JORDER[idx + 1]) if idx < NJ - 1 else None
            emit_rest(j, nch, nmx)
            pend = nxt_pend

        opsp.release()
        ppp.release()
        trp.release()
        workp.release()
        earlyp.release()
        statp.release()
        spsp.release()

    nc.compile()
    return nc


def _prep_inputs(sequence_repr, W_Q, W_K, W_V, mask):
    """Build the 8 per-core input dicts (host-side slicing/permutation)."""
    wqk = np.ascontiguousarray(W_Q @ W_K.T).astype(np.float16)
    wv = np.ascontiguousarray(W_V).astype(np.float16)
    in_maps = []
    meta = []
    for c in range(NCORES):
        b, par = divmod(c, 2)
        pos_blocks = []
        for j in range(NJ):
            pos_blocks += [2 * j + par, 2 * j + 1 - par]
        rows_perm = np.concatenate(
            [np.arange(g * P, (g + 1) * P) for g in pos_blocks])
        xb = sequence_repr[b]
        # xct columns packed [all even-pos blocks | all odd-pos blocks] for
        # 2KB-contiguous DMA lines; xc rows stay position-interleaved
        halves_perm = np.concatenate(
            [np.arange(g * P, (g + 1) * P)
             for g in pos_blocks[0::2] + pos_blocks[1::2]])
        xct = np.ascontiguousarray(xb.T[:, halves_perm]).astype(np.float16)
        xc = np.ascontiguousarray(xb[rows_perm]).astype(np.float16)
        # j-invariant boundary mask: cols [0:128) = own (diagonal) block,
        # [128:256) = other-parity neighbour (all-masked or all-allowed)
        g0, gb0 = par, 1 - par
        qr0 = slice(g0 * P, g0 * P + P)
        madd = np.empty((P, 2 * P), np.float32)
        madd[:, 0:P] = np.where(mask[b, qr0, g0 * P:(g0 + 1) * P],
                                0.0, MASK_FILL)
        madd[:, P:2 * P] = np.where(mask[b, qr0, gb0 * P:(gb0 + 1) * P],
                                    0.0, MASK_FILL)
        in_maps.append({"xct": xct, "xc": xc, "wqk": wqk, "wv": wv,
                        "madd": madd})
        qrows = np.concatenate(
            [np.arange((2 * j + par) * P, (2 * j + par + 1) * P)
             for j in range(NJ)])
        meta.append((b, qrows))
    return in_maps, meta


def run(sequence_repr, W_Q, W_K, W_V, mask, trace=False):
    from concourse.bass_utils import run_bass_kernel_spmd

    if "nc" not in _cache:
        _cache["nc"] = _build_program()
    nc = _cache["nc"]
    in_maps, meta = _prep_inputs(
        np.asarray(sequence_repr, np.float32), np.asarray(W_Q, np.float32),
        np.asarray(W_K, np.float32), np.asarray(W_V, np.float32),
        np.asarray(mask))
    res = run_bass_kernel_spmd(nc, in_maps, core_ids=list(range(NCORES)),
                               trace=trace)
    out = np.empty((B, S, D), np.float32)
    for c in range(NCORES):
        b, qrows = meta[c]
        out[b, qrows] = res.results[c]["out"]
    return out, res


def kernel(**inputs):
    out, _ = run(**inputs)
    return out
